# revision 4
# baseline (speedup 1.0000x reference)
"""2-layer ConvLSTM2D encoder (nn_Encoder_60129542967) on 8 Trainium2 cores.

Sharding: data-parallel over batch (B=8 -> 1 batch/core), conv weights
replicated. Output = final (h, c) of each layer: [2, 2, B, 64, 64, 64].

Device algorithm (per core):
- Feature maps channel-partition: [C, 66*66] zero-padded SBUF buffers.
- U[t%2] = [h0_t (parts 0:64); h1_{t-1} (parts 64:128)], padded fp32r.
- Gate convs as M=128-half matmuls, weights stationary (lhsT [K,128]),
  rhs = shifted window of U, N=512 px, fp32r (full PE rate, ~1.5e-4).
  L1 K-packs its input conv (from h0) + recurrent conv (h1) to K=128.
  L0 recurrent conv K=64 + one K=9 matmul from a pre-shifted x9 buffer.
- Halves: L0 A=[f;i], B=[o;g]; L1 A=[i;f], B=[g;o]. hs-gate weights
  pre-scaled by 0.2 on host; biases (0.2b+0.5 / b) applied on-chip.
- hard_sigmoid = max(x+b', 0) (DVE ts / ACT Relu) then min(.,1) fused
  into the product stt. c' = f*c + i*g via a duplicating-identity
  matmul over t = (S min 1) * V, V = L0:[c;g] / L1:[g;c].
- Phases p=1..11 run L1_{p-1} and L0_p concurrently.
"""

import numpy as np

B, T, H, W, Cin, F = 8, 10, 64, 64, 1, 64
HP = WP = 66
NPIX = H * W
NPAD = HP * WP
NT = 8

IDX_I, IDX_F, IDX_G, IDX_O = 0, 1, 2, 3


def _gc(idx):
    return np.arange(F) + idx * F


A0_COLS = np.concatenate([_gc(IDX_F), _gc(IDX_I)])
B0_COLS = np.concatenate([_gc(IDX_O), _gc(IDX_G)])
A1_COLS = np.concatenate([_gc(IDX_I), _gc(IDX_F)])
B1_COLS = np.concatenate([_gc(IDX_G), _gc(IDX_O)])

_SC = {
    "A0": np.float32(0.2) * np.ones(128, np.float32),
    "B0": np.concatenate([np.full(64, 0.2), np.ones(64)]).astype(np.float32),
    "A1": np.float32(0.2) * np.ones(128, np.float32),
    "B1": np.concatenate([np.ones(64), np.full(64, 0.2)]).astype(np.float32),
}


def pack_weights(Wx0, Wh0, b0, Wx1, Wh1, b1):
    out = {}
    wA0 = np.zeros((9, F, 128), np.float32)
    wB0 = np.zeros((9, F, 128), np.float32)
    wx0 = np.zeros((9, 256), np.float32)
    wA1 = np.zeros((9, 128, 128), np.float32)
    wB1 = np.zeros((9, 128, 128), np.float32)
    for dy in range(3):
        for dx in range(3):
            t = dy * 3 + dx
            wA0[t] = Wh0[dy, dx][:, A0_COLS] * _SC["A0"]
            wB0[t] = Wh0[dy, dx][:, B0_COLS] * _SC["B0"]
            wx0[t, :128] = Wx0[dy, dx, 0][A0_COLS] * _SC["A0"]
            wx0[t, 128:] = Wx0[dy, dx, 0][B0_COLS] * _SC["B0"]
            wA1[t, :F] = Wx1[dy, dx][:, A1_COLS] * _SC["A1"]
            wA1[t, F:] = Wh1[dy, dx][:, A1_COLS] * _SC["A1"]
            wB1[t, :F] = Wx1[dy, dx][:, B1_COLS] * _SC["B1"]
            wB1[t, F:] = Wh1[dy, dx][:, B1_COLS] * _SC["B1"]
    out["wA0"] = np.ascontiguousarray(wA0.transpose(1, 0, 2).reshape(F, 9 * 128))
    out["wB0"] = np.ascontiguousarray(wB0.transpose(1, 0, 2).reshape(F, 9 * 128))
    out["wx0"] = wx0
    out["wA1"] = np.ascontiguousarray(wA1.transpose(1, 0, 2).reshape(128, 9 * 128))
    out["wB1"] = np.ascontiguousarray(wB1.transpose(1, 0, 2).reshape(128, 9 * 128))

    def hsb(b):
        return (0.2 * b + 0.5).astype(np.float32)

    out["biasA0"] = hsb(b0[A0_COLS])[:, None]
    out["biasB0"] = np.concatenate([hsb(b0[B0_COLS[:64]]), b0[B0_COLS[64:]].astype(np.float32)])[:, None]
    out["biasA1"] = hsb(b1[A1_COLS])[:, None]
    out["biasB1"] = np.concatenate([b1[B1_COLS[:64]].astype(np.float32), hsb(b1[B1_COLS[64:]])])[:, None]

    k = np.arange(128)[:, None] % 64
    m = np.arange(128)[None, :] % 64
    out["dup"] = (k == m).astype(np.float32)
    return out


def build_x9(xb):
    x9 = np.zeros((T, 9, HP, WP), np.float32)
    for t in range(T):
        pad = np.pad(xb[t], 1)
        for dy in range(3):
            for dx in range(3):
                x9[t, dy * 3 + dx, 1:65, 1:65] = pad[dy:dy + 64, dx:dx + 64]
    return x9.reshape(T, 9, NPAD)


_EMITTED = {}


def _emit():
    """Build + compile the bass module (cached per process)."""
    if "nc" in _EMITTED:
        return _EMITTED["nc"]

    import concourse.bass as bass
    import concourse.mybir as mybir
    import concourse.tile as tile
    from concourse import bacc

    f32 = mybir.dt.float32
    f32r = mybir.dt.float32r
    Alu = mybir.AluOpType
    Act = mybir.ActivationFunctionType

    nc = bacc.Bacc("TRN2", target_bir_lowering=False, debug=False, num_devices=8)

    dp = {}
    for name, shape in [
        ("wA0", [F, 9 * 128]), ("wB0", [F, 9 * 128]), ("wx0", [9, 256]),
        ("wA1", [128, 9 * 128]), ("wB1", [128, 9 * 128]),
        ("biasA0", [128, 1]), ("biasB0", [128, 1]),
        ("biasA1", [128, 1]), ("biasB1", [128, 1]),
        ("dup", [128, 128]), ("x9", [T, 9, NPAD]), ("zeros", [128, NPAD]),
    ]:
        dp[name] = nc.declare_dram_parameter(name, shape, f32, isOutput=False)
    out_d = nc.declare_dram_parameter("out", [4, 64, NPIX], f32, isOutput=True)

    with tile.TileContext(nc) as tc:
        with (
            tc.tile_pool(name="consts", bufs=1) as consts,
            tc.tile_pool(name="x9p", bufs=2) as x9p,
            tc.tile_pool(name="spool", bufs=3) as spool,
            tc.tile_pool(name="opool", bufs=3) as opool,
            tc.tile_pool(name="tpool", bufs=3) as tpool,
            tc.tile_pool(name="tcpool", bufs=3) as tcpool,
            tc.tile_pool(name="psg", bufs=4, space="PSUM") as psg,
            tc.tile_pool(name="psd", bufs=2, space="PSUM") as psd,
        ):
            # --- constants ---
            w_sb = {}
            for name, parts in [("wA0", F), ("wB0", F), ("wx0", 9),
                                ("wA1", 128), ("wB1", 128)]:
                tl = consts.tile([parts, dp[name].shape[1]], f32r, tag=name, name=name)
                nc.gpsimd.dma_start(out=tl, in_=dp[name][:, :])
                w_sb[name] = tl
            dupw = consts.tile([128, 128], f32r, tag="dup")
            nc.gpsimd.dma_start(out=dupw, in_=dp["dup"][:, :])
            bias_sb = {}
            for name in ("biasA0", "biasB0", "biasA1", "biasB1"):
                tl = consts.tile([128, 1], f32, tag=name, name=name)
                nc.gpsimd.dma_start(out=tl, in_=dp[name][:, :])
                bias_sb[name] = tl

            # --- state buffers ---
            U = [consts.tile([128, NPAD], f32r, tag=f"U{i}", name=f"U{i}") for i in range(2)]
            V = [consts.tile([128, NPIX], f32, tag=f"V{i}", name=f"V{i}") for i in range(2)]
            nc.gpsimd.dma_start(out=U[0][:, :], in_=dp["zeros"][:, :])
            nc.gpsimd.dma_start(out=U[1][:, :], in_=dp["zeros"][:, :])
            nc.vector.memset(V[0][:, :], 0.0)
            nc.gpsimd.memset(V[1][:, :], 0.0)

            def u3(t_ap):
                return t_ap[:, :].rearrange("p (h w) -> p h w", h=HP)

            def emit_unit(layer, t, n, x9t):
                if layer == 0:
                    Uprev, Ucur = U[(t - 1) % 2], U[t % 2]
                    wa, wb = w_sb["wA0"], w_sb["wB0"]
                    bA, bB = bias_sb["biasA0"], bias_sb["biasB0"]
                    K = F
                    gh, oh, ch, hh = slice(64, 128), slice(0, 64), slice(0, 64), slice(0, 64)
                else:
                    Uprev, Ucur = U[t % 2], U[(t + 1) % 2]
                    wa, wb = w_sb["wA1"], w_sb["wB1"]
                    bA, bB = bias_sb["biasA1"], bias_sb["biasB1"]
                    K = 128
                    gh, oh, ch, hh = slice(0, 64), slice(64, 128), slice(64, 128), slice(64, 128)
                Vl = V[layer]
                r0 = n * 8
                UP = u3(Uprev)
                cols = slice(n * 512, (n + 1) * 512)

                A = psg.tile([128, 512], f32, tag="gates")
                Bp = psg.tile([128, 512], f32, tag="gates")
                for ps, w, xc in ((A, wa, slice(0, 128)), (Bp, wb, slice(128, 256))):
                    for tap in range(9):
                        dy, dx = divmod(tap, 3)
                        rhs = UP[0:K, r0 + dy:r0 + dy + 8, dx:dx + 64]
                        nc.tensor.matmul(
                            ps[:, :], w[:, tap * 128:(tap + 1) * 128], rhs,
                            start=(tap == 0),
                            stop=(layer == 1 and tap == 8),
                        )
                    if layer == 0:
                        xr = x9t[:, :].rearrange("p (h w) -> p h w", h=HP)
                        nc.tensor.matmul(
                            ps[:, :], w_sb["wx0"][:, xc],
                            xr[0:9, r0 + 1:r0 + 9, 1:65],
                            start=False, stop=True,
                        )

                S = spool.tile([128, 512], f32, tag="S")
                nc.vector.tensor_scalar(S[:, :], A[:, :], bA[:, 0:1], 0.0,
                                        op0=Alu.add, op1=Alu.max)
                nc.scalar.activation(Vl[gh, cols], Bp[gh, :], Act.Tanh,
                                     bias=bB[gh, 0:1])
                O = opool.tile([128, 512], f32, tag="O")
                nc.scalar.activation(O[oh, :], Bp[oh, :], Act.Relu,
                                     bias=bB[oh, 0:1])
                Tt = tpool.tile([128, 512], f32r, tag="T")
                nc.vector.scalar_tensor_tensor(Tt[:, :], S[:, :], 1.0,
                                               Vl[:, cols],
                                               op0=Alu.min, op1=Alu.mult)
                C = psd.tile([128, 512], f32, tag="dupps")
                nc.tensor.matmul(C[:, :], dupw[:, :], Tt[:, :],
                                 start=True, stop=True)
                nc.vector.tensor_copy(Vl[ch, cols], C[ch, :])
                TC = tcpool.tile([128, 512], f32, tag="TC")
                nc.scalar.activation(TC[ch, :], C[ch, :], Act.Tanh)
                UC = u3(Ucur)
                nc.vector.scalar_tensor_tensor(
                    UC[hh, r0 + 1:r0 + 9, 1:65], O[oh, :], 1.0, TC[ch, :],
                    op0=Alu.min, op1=Alu.mult)

            x9_cur = None
            for p in range(1, T + 2):
                if p <= T:
                    x9t = x9p.tile([9, NPAD], f32r, tag="x9")
                    nc.gpsimd.dma_start(out=x9t, in_=dp["x9"][p - 1])
                else:
                    x9t = None
                for n in range(NT):
                    if p >= 2:
                        emit_unit(1, p - 1, n, None)
                    if p <= T:
                        emit_unit(0, p, n, x9t)

            # --- outputs: h0, c0, h1, c1 each [64, 4096] ---
            U0f = u3(U[T % 2])
            U1f = u3(U[(T + 1) % 2])
            nc.gpsimd.dma_start(out=out_d[0], in_=U0f[0:64, 1:65, 1:65])
            nc.gpsimd.dma_start(out=out_d[1], in_=V[0][0:64, :])
            nc.gpsimd.dma_start(out=out_d[2], in_=U1f[64:128, 1:65, 1:65])
            nc.gpsimd.dma_start(out=out_d[3], in_=V[1][64:128, :])

    nc.compile()
    _EMITTED["nc"] = nc
    return nc


def kernel(x, Wx0, Wh0, b0, Wx1, Wh1, b1):
    from concourse.bass_utils import run_bass_kernel_spmd

    nc = _emit()
    packed = pack_weights(np.asarray(Wx0), np.asarray(Wh0), np.asarray(b0),
                          np.asarray(Wx1), np.asarray(Wh1), np.asarray(b1))
    x = np.asarray(x)
    in_maps = []
    for b in range(B):
        m = dict(packed)
        m["x9"] = build_x9(x[b, :, :, :, 0])
        m["zeros"] = np.zeros((128, NPAD), np.float32)
        in_maps.append(m)
    res = run_bass_kernel_spmd(nc, in_maps, list(range(B)))

    out = np.zeros((2, 2, B, H, W, F), np.float32)
    for b in range(B):
        r = res.results[b]["out"]  # [4, 64, 4096]
        for (l, s), arr in (((0, 0), r[0]), ((0, 1), r[1]),
                            ((1, 0), r[2]), ((1, 1), r[3])):
            out[l, s, b] = arr.T.reshape(H, W, F)
    return out


# revision 23
# speedup vs baseline: 1.2987x; 1.2987x over previous
"""2-layer ConvLSTM2D encoder (nn_Encoder_60129542967) on 8 Trainium2 cores.

Sharding: data-parallel over batch (B=8 -> 1 batch/core), conv weights
replicated. Output = final (h, c) of each layer: [2, 2, B, 64, 64, 64].

Device algorithm (per core):
- Feature maps channel-partition: [C, 66*66] zero-padded SBUF buffers.
- U[t%2] = [h0_t (parts 0:64); h1_{t-1} (parts 64:128)], padded fp32r.
- Gate convs as M=128-half matmuls, weights stationary (lhsT [K,128]),
  rhs = shifted window of U, N=512 px, fp32r (full PE rate, ~1.5e-4).
  L1 K-packs its input conv (from h0) + recurrent conv (h1) to K=128.
  L0 recurrent conv K=64 + one K=9 matmul from a pre-shifted x9 buffer.
- Halves: L0 A=[f;i], B=[o;g]; L1 A=[i;f], B=[g;o]. hs-gate weights
  pre-scaled by 0.2 on host; biases (0.2b+0.5 / b) applied on-chip.
- hard_sigmoid = max(x+b', 0) (DVE ts / ACT Relu) then min(.,1) fused
  into the product stt. c' = f*c + i*g via a duplicating-identity
  matmul over t = (S min 1) * V, V = L0:[c;g] / L1:[g;c].
- Phases p=1..11 run L1_{p-1} and L0_p concurrently.
"""

import numpy as np

B, T, H, W, Cin, F = 8, 10, 64, 64, 1, 64
HP = WP = 66
NPIX = H * W
NPAD = HP * WP
NT = 8

IDX_I, IDX_F, IDX_G, IDX_O = 0, 1, 2, 3
WCAT_LAYOUT = {
    "wEA0": (73, 0, 128), "wEB0": (73, 128, 128),
    "wSA0": (64, 256, 256), "wSB0": (64, 512, 256),
    "wPA0": (128, 768, 384), "wPB0": (128, 1152, 384),
    "wA1": (128, 1536, 1152), "wB1": (128, 2688, 1152),
    "dup": (128, 3840, 128),
}


def _gc(idx):
    return np.arange(F) + idx * F


A0_COLS = np.concatenate([_gc(IDX_F), _gc(IDX_I)])
B0_COLS = np.concatenate([_gc(IDX_O), _gc(IDX_G)])
A1_COLS = np.concatenate([_gc(IDX_I), _gc(IDX_F)])
B1_COLS = np.concatenate([_gc(IDX_G), _gc(IDX_O)])

_SC = {
    "A0": np.float32(0.2) * np.ones(128, np.float32),
    "B0": np.concatenate([np.full(64, 0.2), np.ones(64)]).astype(np.float32),
    "A1": np.float32(0.2) * np.ones(128, np.float32),
    "B1": np.concatenate([np.ones(64), np.full(64, 0.2)]).astype(np.float32),
}


def pack_weights(Wx0, Wh0, b0, Wx1, Wh1, b1):
    out = {}
    wA0 = np.zeros((9, F, 128), np.float32)
    wB0 = np.zeros((9, F, 128), np.float32)
    wx0 = np.zeros((9, 256), np.float32)
    wA1 = np.zeros((9, 128, 128), np.float32)
    wB1 = np.zeros((9, 128, 128), np.float32)
    for dy in range(3):
        for dx in range(3):
            t = dy * 3 + dx
            wA0[t] = Wh0[dy, dx][:, A0_COLS] * _SC["A0"]
            wB0[t] = Wh0[dy, dx][:, B0_COLS] * _SC["B0"]
            wx0[t, :128] = Wx0[dy, dx, 0][A0_COLS] * _SC["A0"]
            wx0[t, 128:] = Wx0[dy, dx, 0][B0_COLS] * _SC["B0"]
            wA1[t, :F] = Wx1[dy, dx][:, A1_COLS] * _SC["A1"]
            wA1[t, F:] = Wh1[dy, dx][:, A1_COLS] * _SC["A1"]
            wB1[t, :F] = Wx1[dy, dx][:, B1_COLS] * _SC["B1"]
            wB1[t, F:] = Wh1[dy, dx][:, B1_COLS] * _SC["B1"]
    # L0 tap-paired layouts: P = (dy, dx=0)+(dy, dx=1) pairs on K=128 via
    # the flat-shift-by-1 duplicate; (0,2),(1,2) stay K=64 singles; the
    # (2,2) tap is fused with the 9-tap x-conv into one K=73 matmul (wE).
    for half, w in (("A0", wA0), ("B0", wB0)):
        wp = np.zeros((3, 128, 128), np.float32)
        ws = np.zeros((2, F, 128), np.float32)
        for dy in range(3):
            wp[dy, :F] = w[dy * 3 + 0]
            wp[dy, F:] = w[dy * 3 + 1]
        for dy in range(2):
            ws[dy] = w[dy * 3 + 2]
        out[f"wP{half}"] = np.ascontiguousarray(
            wp.transpose(1, 0, 2).reshape(128, 3 * 128))
        out[f"wS{half}"] = np.ascontiguousarray(
            ws.transpose(1, 0, 2).reshape(F, 2 * 128))
        we = np.zeros((73, 128), np.float32)
        we[:F] = w[2 * 3 + 2]
        xc = slice(0, 128) if half == "A0" else slice(128, 256)
        we[F:] = wx0[:, xc]
        out[f"wE{half}"] = we
    out["wA1"] = np.ascontiguousarray(wA1.transpose(1, 0, 2).reshape(128, 9 * 128))
    out["wB1"] = np.ascontiguousarray(wB1.transpose(1, 0, 2).reshape(128, 9 * 128))

    def hsb(b):
        return (0.2 * b + 0.5).astype(np.float32)

    out["biasA0"] = hsb(b0[A0_COLS])[:, None]
    out["biasB0"] = np.concatenate([hsb(b0[B0_COLS[:64]]), b0[B0_COLS[64:]].astype(np.float32)])[:, None]
    out["biasA1"] = hsb(b1[A1_COLS])[:, None]
    out["biasB1"] = np.concatenate([b1[B1_COLS[:64]].astype(np.float32), hsb(b1[B1_COLS[64:]])])[:, None]

    k = np.arange(128)[:, None] % 64
    m = np.arange(128)[None, :] % 64
    out["dup"] = (k == m).astype(np.float32)

    # one slab for a single casting DMA: layout per WCAT_LAYOUT
    wcat = np.zeros((128, 3968), np.float32)
    for nm, (parts, off, width) in WCAT_LAYOUT.items():
        a = out.pop(nm)
        assert a.shape == (parts, width), (nm, a.shape)
        wcat[:parts, off:off + width] = a
    out["wcat"] = wcat
    return out


def build_x9(xb):
    # x9[t, tap] aligned for reads at the (dy=2, dx=2) conv window:
    # x9[t, tap, (y+2)*66 + (x+2)] = x_t[y+dy-1, x+dx-1]
    x9 = np.zeros((T, 9, HP, WP), np.float32)
    for t in range(T):
        pad = np.pad(xb[t], 1)
        for dy in range(3):
            for dx in range(3):
                x9[t, dy * 3 + dx, 2:66, 2:66] = pad[dy:dy + 64, dx:dx + 64]
    return x9.reshape(T, 9, NPAD)


_EMITTED = {}


def _emit():
    """Build + compile the bass module (cached per process)."""
    if "nc" in _EMITTED:
        return _EMITTED["nc"]

    import concourse.bass as bass
    import concourse.mybir as mybir
    import concourse.tile as tile
    from concourse import bacc

    f32 = mybir.dt.float32
    f32r = mybir.dt.float32r
    Alu = mybir.AluOpType
    Act = mybir.ActivationFunctionType

    nc = bacc.Bacc("TRN2", target_bir_lowering=False, debug=False, num_devices=8)

    dp = {}
    for name, shape in [
        ("wcat", [128, 3968]),
        ("biasA0", [128, 1]), ("biasB0", [128, 1]),
        ("biasA1", [128, 1]), ("biasB1", [128, 1]),
        ("x9", [T, 9, NPAD]), ("zeros", [128, NPAD]),
    ]:
        dp[name] = nc.declare_dram_parameter(name, shape, f32, isOutput=False)
    out_d = nc.declare_dram_parameter("out", [4, 64, NPIX], f32, isOutput=True)

    with tile.TileContext(nc) as tc:
        with (
            tc.tile_pool(name="consts", bufs=1) as consts,
            tc.tile_pool(name="dpool", bufs=2) as dpool,
            tc.tile_pool(name="epool", bufs=2) as epool,
            tc.tile_pool(name="spool", bufs=3) as spool,
            tc.tile_pool(name="opool", bufs=3) as opool,
            tc.tile_pool(name="tpool", bufs=3) as tpool,
            tc.tile_pool(name="tcpool", bufs=3) as tcpool,
            tc.tile_pool(name="psg", bufs=4, space="PSUM") as psg,
            tc.tile_pool(name="psd", bufs=2, space="PSUM") as psd,
        ):
            # --- constants ---
            bias_sb = {}
            for name in ("biasA0", "biasB0", "biasA1", "biasB1"):
                tl = consts.tile([128, 1], f32, tag=name, name=name)
                nc.gpsimd.dma_start(out=tl, in_=dp[name][:, :])
                bias_sb[name] = tl
            # fp32r weights in one slab, loaded phase-1-critical part first
            wcat = consts.tile([128, 3968], f32r, tag="wcat", name="wcat")
            nc.gpsimd.dma_start(out=wcat[:, 0:768], in_=dp["wcat"][:, 0:768])
            w_sb = {}
            for nm, (parts, off, width) in WCAT_LAYOUT.items():
                w_sb[nm] = wcat[0:parts, off:off + width]
            dupw = w_sb["dup"]

            # --- state buffers ---
            U = [consts.tile([128, NPAD], f32r, tag=f"U{i}", name=f"U{i}") for i in range(2)]
            V = [consts.tile([128, NPIX], f32, tag=f"V{i}", name=f"V{i}") for i in range(2)]
            zr = dp["zeros"][:, :].bitcast(f32r)
            nc.sync.dma_start(out=U[0][:, :], in_=zr)
            nc.scalar.dma_start(out=U[1][:, :], in_=zr)
            nc.vector.memset(V[0][:, :], 0.0)
            nc.vector.memset(V[1][:, :], 0.0)

            def u3(t_ap):
                return t_ap[:, :].rearrange("p (h w) -> p h w", h=HP)

            def emit_unit(layer, t, n, D0=None, E0=None):
                if layer == 0:
                    Uprev, Ucur = U[(t - 1) % 2], U[t % 2]
                    bA, bB = bias_sb["biasA0"], bias_sb["biasB0"]
                    gh, oh, ch, hh = slice(64, 128), slice(0, 64), slice(0, 64), slice(0, 64)
                else:
                    Uprev, Ucur = U[t % 2], U[(t + 1) % 2]
                    bA, bB = bias_sb["biasA1"], bias_sb["biasB1"]
                    gh, oh, ch, hh = slice(0, 64), slice(64, 128), slice(64, 128), slice(64, 128)
                Vl = V[layer]
                r0 = n * 8
                cols = slice(n * 512, (n + 1) * 512)

                A = psg.tile([128, 512], f32, tag="gates")
                Bp = psg.tile([128, 512], f32, tag="gates")
                if layer == 1:
                    UP = u3(Uprev)
                    for ps, w in ((A, w_sb["wA1"]), (Bp, w_sb["wB1"])):
                        for tap in range(9):
                            dy, dx = divmod(tap, 3)
                            rhs = UP[0:128, r0 + dy:r0 + dy + 8, dx:dx + 64]
                            nc.tensor.matmul(
                                ps[:, :], w[:, tap * 128:(tap + 1) * 128], rhs,
                                start=(tap == 0), stop=(tap == 8),
                            )
                else:
                    # P(dy) covers (dy,0)+(dy,1) on K=128 via the
                    # flat-shift-by-1 duplicate; (0,2),(1,2) K=64 singles read
                    # the duplicate's lower half; (2,2) rides the K=73 E-matmul
                    # together with the 9-tap x-conv. At t==1 h0_0 == 0 so
                    # only the x-conv rows of the E-matmul are emitted.
                    E03 = u3(E0)
                    if t == 1:
                        for ps, hf in ((A, "A0"), (Bp, "B0")):
                            nc.tensor.matmul(
                                ps[:, :], w_sb[f"wE{hf}"][64:73, :],
                                E03[64:73, r0 + 2:r0 + 10, 2:66],
                                start=True, stop=True,
                            )
                    else:
                        D03 = u3(D0)
                        for ps, hf in ((A, "A0"), (Bp, "B0")):
                            for dy in range(3):
                                nc.tensor.matmul(
                                    ps[:, :],
                                    w_sb[f"wP{hf}"][:, dy * 128:(dy + 1) * 128],
                                    D03[0:128, r0 + dy:r0 + dy + 8, 0:64],
                                    start=(dy == 0), stop=False,
                                )
                            for dy in range(2):
                                nc.tensor.matmul(
                                    ps[:, :],
                                    w_sb[f"wS{hf}"][:, dy * 128:(dy + 1) * 128],
                                    D03[0:64, r0 + dy:r0 + dy + 8, 2:66],
                                    start=False, stop=False,
                                )
                            nc.tensor.matmul(
                                ps[:, :], w_sb[f"wE{hf}"][:, :],
                                E03[0:73, r0 + 2:r0 + 10, 2:66],
                                start=False, stop=True,
                            )

                S = spool.tile([128, 512], f32, tag="S")
                nc.vector.tensor_scalar(S[:, :], A[:, :], bA[:, 0:1], 0.0,
                                        op0=Alu.add, op1=Alu.max)
                nc.scalar.activation(Vl[gh, cols], Bp[gh, :], Act.Tanh,
                                     bias=bB[gh, 0:1])
                O = opool.tile([128, 512], f32, tag="O")
                nc.scalar.activation(O[oh, :], Bp[oh, :], Act.Relu,
                                     bias=bB[oh, 0:1])
                Tt = tpool.tile([128, 512], f32r, tag="T")
                nc.vector.scalar_tensor_tensor(Tt[:, :], S[:, :], 1.0,
                                               Vl[:, cols],
                                               op0=Alu.min, op1=Alu.mult)
                C = psd.tile([128, 512], f32, tag="dupps")
                nc.tensor.matmul(C[:, :], dupw[:, :], Tt[:, :],
                                 start=True, stop=True)
                nc.vector.tensor_copy(Vl[ch, cols], C[ch, :])
                TC = tcpool.tile([128, 512], f32, tag="TC")
                nc.scalar.activation(TC[ch, :], C[ch, :], Act.Tanh)
                UC = u3(Ucur)
                nc.vector.scalar_tensor_tensor(
                    UC[hh, r0 + 1:r0 + 9, 1:65], O[oh, :], 1.0, TC[ch, :],
                    op0=Alu.min, op1=Alu.mult)

            def emit_dchunk(Ut, D_next, E_next, n):
                """Copy h0 chunk n (just written into Ut) into D_next
                (plain + flat-shift-by-1 planes) and E_next (plain)."""
                r0 = n * 8
                lo = 0 if n == 0 else (r0 + 1) * HP
                hi = NPAD if n == NT - 1 else (r0 + 9) * HP
                nc.sync.dma_start(out=D_next[0:64, lo:hi], in_=Ut[0:64, lo:hi])
                nc.sync.dma_start(out=E_next[0:64, lo:hi], in_=Ut[0:64, lo:hi])
                l2 = 0 if n == 0 else (r0 + 1) * HP - 1
                h2 = NPAD - 1 if n == NT - 1 else (r0 + 9) * HP - 1
                nc.scalar.dma_start(out=D_next[64:128, l2:h2],
                                    in_=Ut[0:64, l2 + 1:h2 + 1])

            U0f = u3(U[T % 2])
            U1f = u3(U[(T + 1) % 2])
            o_h0 = out_d[0].bitcast(f32r).rearrange("p (h w) -> p h w", h=H)
            o_h1 = out_d[2].bitcast(f32r).rearrange("p (h w) -> p h w", h=H)

            E_cur = epool.tile([73, NPAD], f32r, tag="E", name="Einit")
            # phase-1 x9 rows, chunked so unit n can start after chunk n
            for n in range(NT):
                lo, hi = (n * 8 + 2) * HP, (n * 8 + 10) * HP
                nc.gpsimd.dma_start(out=E_cur[64:73, lo:hi],
                                    in_=dp["x9"][0][:, lo:hi])
            nc.gpsimd.dma_start(out=wcat[:, 768:3968], in_=dp["wcat"][:, 768:3968])
            D_cur = None
            for p in range(1, T + 2):
                if p <= T:
                    if p < T:
                        D_next = dpool.tile([128, NPAD], f32r, tag="D0", name="D0")
                        E_next = epool.tile([73, NPAD], f32r, tag="E", name="E0")
                        # prefetch next phase's x-conv rows during this phase
                        nc.gpsimd.dma_start(out=E_next[64:73, :], in_=dp["x9"][p])
                    else:
                        D_next = E_next = None
                    for n in range(NT):
                        emit_unit(0, p, n, D_cur, E_cur)
                        if D_next is not None:
                            emit_dchunk(U[p % 2], D_next, E_next, n)
                        if p == T:
                            # stream final h0/c0 out as they are produced
                            r0 = n * 8
                            nc.sync.dma_start(
                                out=o_h0[:, r0:r0 + 8, :],
                                in_=U0f[0:64, r0 + 1:r0 + 9, 1:65])
                            nc.scalar.dma_start(
                                out=out_d[1][:, n * 512:(n + 1) * 512],
                                in_=V[0][0:64, n * 512:(n + 1) * 512])
                else:
                    D_next = E_next = None
                if p >= 2:
                    for n in range(NT):
                        emit_unit(1, p - 1, n)
                        if p == T + 1:
                            r0 = n * 8
                            nc.sync.dma_start(
                                out=o_h1[:, r0:r0 + 8, :],
                                in_=U1f[64:128, r0 + 1:r0 + 9, 1:65])
                            nc.scalar.dma_start(
                                out=out_d[3][:, n * 512:(n + 1) * 512],
                                in_=V[1][64:128, n * 512:(n + 1) * 512])
                D_cur, E_cur = D_next, E_next

    nc.compile()
    _EMITTED["nc"] = nc
    return nc


def kernel(x, Wx0, Wh0, b0, Wx1, Wh1, b1):
    from concourse.bass_utils import run_bass_kernel_spmd

    nc = _emit()
    packed = pack_weights(np.asarray(Wx0), np.asarray(Wh0), np.asarray(b0),
                          np.asarray(Wx1), np.asarray(Wh1), np.asarray(b1))
    x = np.asarray(x)
    in_maps = []
    for b in range(B):
        m = dict(packed)
        m["x9"] = build_x9(x[b, :, :, :, 0])
        m["zeros"] = np.zeros((128, NPAD), np.float32)
        in_maps.append(m)
    res = run_bass_kernel_spmd(nc, in_maps, list(range(B)))

    out = np.zeros((2, 2, B, H, W, F), np.float32)
    for b in range(B):
        r = res.results[b]["out"]  # [4, 64, 4096]
        for (l, s), arr in (((0, 0), r[0]), ((0, 1), r[1]),
                            ((1, 0), r[2]), ((1, 1), r[3])):
            out[l, s, b] = arr.T.reshape(H, W, F)
    return out


# revision 38
# speedup vs baseline: 1.4516x; 1.1178x over previous
"""2-layer ConvLSTM2D encoder (nn_Encoder_60129542967) on 8 Trainium2 cores.

Sharding: data-parallel over batch (B=8 -> 1 batch/core), conv weights
replicated. Output = final (h, c) of each layer: [2, 2, B, 64, 64, 64].

Device algorithm (per core):
- Feature maps channel-partition: [C, 66*66] zero-padded SBUF buffers.
- U[t%2] = [h0_t (parts 0:64); h1_{t-1} (parts 64:128)], padded fp32r.
- Gate convs as M=128-half matmuls, weights stationary (lhsT [K,128]),
  rhs = shifted window of U, N=512 px, fp32r (full PE rate, ~1.5e-4).
  L1 K-packs its input conv (from h0) + recurrent conv (h1) to K=128.
  L0 recurrent conv K=64 + one K=9 matmul from a pre-shifted x9 buffer.
- Halves: L0 A=[f;i], B=[o;g]; L1 A=[i;f], B=[g;o]. hs-gate weights
  pre-scaled by 0.2 on host; biases (0.2b+0.5 / b) applied on-chip.
- hard_sigmoid = max(x+b', 0) (DVE ts / ACT Relu) then min(.,1) fused
  into the product stt. c' = f*c + i*g via a duplicating-identity
  matmul over t = (S min 1) * V, V = L0:[c;g] / L1:[g;c].
- Phases p=1..11 run L1_{p-1} and L0_p concurrently.
"""

import numpy as np

B, T, H, W, Cin, F = 8, 10, 64, 64, 1, 64
HP = WP = 66
NPIX = H * W
NPAD = HP * WP
NT = 8

IDX_I, IDX_F, IDX_G, IDX_O = 0, 1, 2, 3
WCAT_LAYOUT = {
    "wEA0": (73, 0, 128), "wEB0": (73, 128, 128),
    "wA1": (128, 256, 1152), "wB1": (128, 1408, 1152),
    "wQA0": (128, 2560, 128), "wQB0": (128, 2688, 128),
    "wPA0": (128, 2816, 384), "wPB0": (128, 3200, 384),
}


def _gc(idx):
    return np.arange(F) + idx * F


A0_COLS = np.concatenate([_gc(IDX_F), _gc(IDX_I)])
B0_COLS = np.concatenate([_gc(IDX_O), _gc(IDX_G)])
A1_COLS = np.concatenate([_gc(IDX_I), _gc(IDX_F)])
B1_COLS = np.concatenate([_gc(IDX_G), _gc(IDX_O)])

_SC = {
    "A0": np.float32(0.2) * np.ones(128, np.float32),
    "B0": np.concatenate([np.full(64, 0.2), np.ones(64)]).astype(np.float32),
    "A1": np.float32(0.2) * np.ones(128, np.float32),
    "B1": np.concatenate([np.ones(64), np.full(64, 0.2)]).astype(np.float32),
}


def pack_weights(Wx0, Wh0, b0, Wx1, Wh1, b1):
    out = {}
    wA0 = np.zeros((9, F, 128), np.float32)
    wB0 = np.zeros((9, F, 128), np.float32)
    wx0 = np.zeros((9, 256), np.float32)
    wA1 = np.zeros((9, 128, 128), np.float32)
    wB1 = np.zeros((9, 128, 128), np.float32)
    for dy in range(3):
        for dx in range(3):
            t = dy * 3 + dx
            wA0[t] = Wh0[dy, dx][:, A0_COLS] * _SC["A0"]
            wB0[t] = Wh0[dy, dx][:, B0_COLS] * _SC["B0"]
            wx0[t, :128] = Wx0[dy, dx, 0][A0_COLS] * _SC["A0"]
            wx0[t, 128:] = Wx0[dy, dx, 0][B0_COLS] * _SC["B0"]
            wA1[t, :F] = Wx1[dy, dx][:, A1_COLS] * _SC["A1"]
            wA1[t, F:] = Wh1[dy, dx][:, A1_COLS] * _SC["A1"]
            wB1[t, :F] = Wx1[dy, dx][:, B1_COLS] * _SC["B1"]
            wB1[t, F:] = Wh1[dy, dx][:, B1_COLS] * _SC["B1"]
    # L0 tap-paired layouts: P = (dy, dx=0)+(dy, dx=1) pairs on K=128 via
    # the flat-shift-by-1 duplicate; (0,2),(1,2) stay K=64 singles; the
    # (2,2) tap is fused with the 9-tap x-conv into one K=73 matmul (wE).
    for half, w in (("A0", wA0), ("B0", wB0)):
        wp = np.zeros((3, 128, 128), np.float32)
        ws = np.zeros((2, F, 128), np.float32)
        for dy in range(3):
            wp[dy, :F] = w[dy * 3 + 0]
            wp[dy, F:] = w[dy * 3 + 1]
        for dy in range(2):
            ws[dy] = w[dy * 3 + 2]
        wq = np.zeros((128, 128), np.float32)
        wq[:F] = w[0 * 3 + 2]
        wq[F:] = w[1 * 3 + 2]
        out[f"wQ{half}"] = wq
        out[f"wP{half}"] = np.ascontiguousarray(
            wp.transpose(1, 0, 2).reshape(128, 3 * 128))
        we = np.zeros((73, 128), np.float32)
        we[:F] = w[2 * 3 + 2]
        xc = slice(0, 128) if half == "A0" else slice(128, 256)
        we[F:] = wx0[:, xc]
        out[f"wE{half}"] = we
    out["wA1"] = np.ascontiguousarray(wA1.transpose(1, 0, 2).reshape(128, 9 * 128))
    out["wB1"] = np.ascontiguousarray(wB1.transpose(1, 0, 2).reshape(128, 9 * 128))

    def hsb(b):
        return (0.2 * b + 0.5).astype(np.float32)

    out["biasA0"] = hsb(b0[A0_COLS])[:, None]
    out["biasB0"] = np.concatenate([hsb(b0[B0_COLS[:64]]), b0[B0_COLS[64:]].astype(np.float32)])[:, None]
    out["biasA1"] = hsb(b1[A1_COLS])[:, None]
    out["biasB1"] = np.concatenate([b1[B1_COLS[:64]].astype(np.float32), hsb(b1[B1_COLS[64:]])])[:, None]

    # one slab for a single casting DMA: layout per WCAT_LAYOUT
    wcat = np.zeros((128, 3584), np.float32)
    for nm, (parts, off, width) in WCAT_LAYOUT.items():
        a = out.pop(nm)
        assert a.shape == (parts, width), (nm, a.shape)
        wcat[:parts, off:off + width] = a
    out["wcat"] = wcat
    return out


def build_x9(xb):
    # x9[t, tap] aligned for reads at the (dy=2, dx=2) conv window:
    # x9[t, tap, (y+2)*66 + (x+2)] = x_t[y+dy-1, x+dx-1]
    x9 = np.zeros((T, 9, HP, WP), np.float32)
    for t in range(T):
        pad = np.pad(xb[t], 1)
        for dy in range(3):
            for dx in range(3):
                x9[t, dy * 3 + dx, 2:66, 2:66] = pad[dy:dy + 64, dx:dx + 64]
    return x9.reshape(T, 9, NPAD)


_EMITTED = {}


def _emit():
    """Build + compile the bass module (cached per process)."""
    if "nc" in _EMITTED:
        return _EMITTED["nc"]

    import concourse.bass as bass
    import concourse.mybir as mybir
    import concourse.tile as tile
    from concourse import bacc

    f32 = mybir.dt.float32
    f32r = mybir.dt.float32r
    Alu = mybir.AluOpType
    Act = mybir.ActivationFunctionType

    nc = bacc.Bacc("TRN2", target_bir_lowering=False, debug=False, num_devices=8)

    dp = {}
    for name, shape in [
        ("wcat", [128, 3584]),
        ("biasA0", [128, 1]), ("biasB0", [128, 1]),
        ("biasA1", [128, 1]), ("biasB1", [128, 1]),
        ("x9", [T, 9, NPAD]), ("zeros", [128, NPAD]),
    ]:
        dp[name] = nc.declare_dram_parameter(name, shape, f32, isOutput=False)
    out_d = nc.declare_dram_parameter("out", [4, 64, NPIX], f32, isOutput=True)

    with tile.TileContext(nc) as tc:
        with (
            tc.tile_pool(name="consts", bufs=1) as consts,
            tc.tile_pool(name="dpool", bufs=2) as dpool,
            tc.tile_pool(name="gpool", bufs=2) as gpool,
            tc.tile_pool(name="epool", bufs=2) as epool,
            tc.tile_pool(name="spool", bufs=3) as spool,
            tc.tile_pool(name="opool", bufs=2) as opool,
            tc.tile_pool(name="tpool", bufs=3) as tpool,
            tc.tile_pool(name="tcpool", bufs=2) as tcpool,
            tc.tile_pool(name="xpool", bufs=2) as xpool,
            tc.tile_pool(name="psg", bufs=8, space="PSUM") as psg,
        ):
            # --- constants ---
            # fp32r weights in one slab; phase-1 needs only wE (first 256
            # cols), so that slice loads first, then phase-1's x9 chunks,
            # then everything else in need-order
            wcat = consts.tile([128, 3584], f32r, tag="wcat", name="wcat")
            nc.gpsimd.dma_start(out=wcat[:, 0:256], in_=dp["wcat"][:, 0:256])
            bias_sb = {}
            w_sb = {}
            for nm, (parts, off, width) in WCAT_LAYOUT.items():
                w_sb[nm] = wcat[0:parts, off:off + width]

            # --- state buffers ---
            U = [consts.tile([128, NPAD], f32r, tag=f"U{i}", name=f"U{i}") for i in range(2)]
            V = [consts.tile([128, NPIX], f32, tag=f"V{i}", name=f"V{i}") for i in range(2)]
            zr = dp["zeros"][:, :].bitcast(f32r)
            nc.sync.dma_start(out=U[0][:, :], in_=zr)
            nc.scalar.dma_start(out=U[1][:, :], in_=zr)
            nc.vector.memset(V[0][:, :], 0.0)
            nc.vector.memset(V[1][:, :], 0.0)

            def u3(t_ap):
                return t_ap[:, :].rearrange("p (h w) -> p h w", h=HP)

            def emit_unit(layer, t, n, D0=None, E0=None, G0=None):
                if layer == 0:
                    Uprev, Ucur = U[(t - 1) % 2], U[t % 2]
                    bA, bB = bias_sb["biasA0"], bias_sb["biasB0"]
                    gh, oh, ch, hh = slice(64, 128), slice(0, 64), slice(0, 64), slice(0, 64)
                else:
                    Uprev, Ucur = U[t % 2], U[(t + 1) % 2]
                    bA, bB = bias_sb["biasA1"], bias_sb["biasB1"]
                    gh, oh, ch, hh = slice(0, 64), slice(64, 128), slice(64, 128), slice(64, 128)
                Vl = V[layer]
                r0 = n * 8
                cols = slice(n * 512, (n + 1) * 512)

                A = psg.tile([128, 512], f32, tag="gates")
                Bp = psg.tile([128, 512], f32, tag="gates")
                if layer == 1:
                    UP = u3(Uprev)
                    for ps, w in ((A, w_sb["wA1"]), (Bp, w_sb["wB1"])):
                        for tap in range(9):
                            dy, dx = divmod(tap, 3)
                            rhs = UP[0:128, r0 + dy:r0 + dy + 8, dx:dx + 64]
                            nc.tensor.matmul(
                                ps[:, :], w[:, tap * 128:(tap + 1) * 128], rhs,
                                start=(tap == 0), stop=(tap == 8),
                            )
                else:
                    # P(dy) covers (dy,0)+(dy,1) on K=128 via the
                    # flat-shift-by-1 duplicate; (0,2),(1,2) K=64 singles read
                    # the duplicate's lower half; (2,2) rides the K=73 E-matmul
                    # together with the 9-tap x-conv. At t==1 h0_0 == 0 so
                    # only the x-conv rows of the E-matmul are emitted.
                    E03 = u3(E0)
                    if t == 1:
                        for ps, hf in ((A, "A0"), (Bp, "B0")):
                            nc.tensor.matmul(
                                ps[:, :], w_sb[f"wE{hf}"][64:73, :],
                                E03[64:73, r0 + 2:r0 + 10, 2:66],
                                start=True, stop=True,
                            )
                    else:
                        D03 = u3(D0)
                        G03 = u3(G0)
                        for ps, hf in ((A, "A0"), (Bp, "B0")):
                            for dy in range(3):
                                nc.tensor.matmul(
                                    ps[:, :],
                                    w_sb[f"wP{hf}"][:, dy * 128:(dy + 1) * 128],
                                    D03[0:128, r0 + dy:r0 + dy + 8, 0:64],
                                    start=(dy == 0), stop=False,
                                )
                            nc.tensor.matmul(
                                ps[:, :], w_sb[f"wQ{hf}"][:, :],
                                G03[0:128, r0:r0 + 8, 2:66],
                                start=False, stop=False,
                            )
                            nc.tensor.matmul(
                                ps[:, :], w_sb[f"wE{hf}"][:, :],
                                E03[0:73, r0 + 2:r0 + 10, 2:66],
                                start=False, stop=True,
                            )

                S = spool.tile([128, 512], f32, tag="S")
                nc.vector.tensor_scalar(S[:, :], A[:, :], bA[:, 0:1], 0.0,
                                        op0=Alu.add, op1=Alu.max)
                nc.scalar.activation(Vl[gh, cols], Bp[gh, :], Act.Tanh,
                                     bias=bB[gh, 0:1])
                O = opool.tile([128, 512], f32, tag="O")
                nc.scalar.activation(O[oh, :], Bp[oh, :], Act.Relu,
                                     bias=bB[oh, 0:1])
                Tt = tpool.tile([128, 512], f32, tag="T")
                nc.vector.scalar_tensor_tensor(Tt[:, :], S[:, :], 1.0,
                                               Vl[:, cols],
                                               op0=Alu.min, op1=Alu.mult)
                # c' = f*c + i*g: move the off-half product onto ch's
                # partitions by DMA, then a plain DVE add into the c state.
                Xt = xpool.tile([128, 512], f32, tag="X")
                oh_half = slice(64, 128) if ch == slice(0, 64) else slice(0, 64)
                nc.sync.dma_start(out=Xt[ch, :], in_=Tt[oh_half, :])
                nc.vector.tensor_tensor(Vl[ch, cols], Tt[ch, :], Xt[ch, :],
                                        op=Alu.add)
                TC = tcpool.tile([128, 512], f32, tag="TC")
                nc.scalar.activation(TC[ch, :], Vl[ch, cols], Act.Tanh)
                UC = u3(Ucur)
                nc.vector.scalar_tensor_tensor(
                    UC[hh, r0 + 1:r0 + 9, 1:65], O[oh, :], 1.0, TC[ch, :],
                    op0=Alu.min, op1=Alu.mult)

            def emit_dchunk(Ut, D_next, E_next, G_next, n):
                """Copy h0 chunk n (just written into Ut) into D_next
                (plain + flat-shift-by-1 planes), E_next (plain) and
                G_next (plain + flat-shift-by-66 planes)."""
                r0 = n * 8
                lo = 0 if n == 0 else (r0 + 1) * HP
                hi = NPAD if n == NT - 1 else (r0 + 9) * HP
                nc.sync.dma_start(out=D_next[0:64, lo:hi], in_=Ut[0:64, lo:hi])
                nc.sync.dma_start(out=E_next[0:64, lo:hi], in_=Ut[0:64, lo:hi])
                nc.scalar.dma_start(out=G_next[0:64, lo:hi], in_=Ut[0:64, lo:hi])
                l2 = 0 if n == 0 else (r0 + 1) * HP - 1
                h2 = NPAD - 1 if n == NT - 1 else (r0 + 9) * HP - 1
                nc.scalar.dma_start(out=D_next[64:128, l2:h2],
                                    in_=Ut[0:64, l2 + 1:h2 + 1])
                l6 = 0 if n == 0 else (r0 + 1) * HP - 66
                h6 = NPAD - 66 if n == NT - 1 else (r0 + 9) * HP - 66
                nc.sync.dma_start(out=G_next[64:128, l6:h6],
                                  in_=Ut[0:64, l6 + 66:h6 + 66])

            U0f = u3(U[T % 2])
            U1f = u3(U[(T + 1) % 2])
            o_h0 = out_d[0].bitcast(f32r).rearrange("p (h w) -> p h w", h=H)
            o_h1 = out_d[2].bitcast(f32r).rearrange("p (h w) -> p h w", h=H)

            E_cur = epool.tile([73, NPAD], f32r, tag="E", name="Einit")
            # phase-1 x9 rows, chunked so unit n can start after chunk n
            for n in range(NT):
                lo, hi = (n * 8 + 2) * HP, (n * 8 + 10) * HP
                nc.gpsimd.dma_start(out=E_cur[64:73, lo:hi],
                                    in_=dp["x9"][0][:, lo:hi])
            for name in ("biasA0", "biasB0", "biasA1", "biasB1"):
                tl = consts.tile([128, 1], f32, tag=name, name=name)
                nc.gpsimd.dma_start(out=tl, in_=dp[name][:, :])
                bias_sb[name] = tl
            nc.gpsimd.dma_start(out=wcat[:, 2560:3584], in_=dp["wcat"][:, 2560:3584])
            nc.gpsimd.dma_start(out=wcat[:, 256:2560], in_=dp["wcat"][:, 256:2560])
            D_cur = G_cur = None
            for p in range(1, T + 2):
                if p <= T:
                    if p < T:
                        D_next = dpool.tile([128, NPAD], f32r, tag="D0", name="D0")
                        E_next = epool.tile([73, NPAD], f32r, tag="E", name="E0")
                        G_next = gpool.tile([128, NPAD], f32r, tag="G", name="G0")
                        # prefetch next phase's x-conv rows during this phase
                        nc.gpsimd.dma_start(out=E_next[64:73, :], in_=dp["x9"][p])
                    else:
                        D_next = E_next = G_next = None
                    for n in range(NT):
                        emit_unit(0, p, n, D_cur, E_cur, G_cur)
                        if D_next is not None:
                            emit_dchunk(U[p % 2], D_next, E_next, G_next, n)
                        if p == T:
                            # stream final h0/c0 out as they are produced
                            r0 = n * 8
                            nc.sync.dma_start(
                                out=o_h0[:, r0:r0 + 8, :],
                                in_=U0f[0:64, r0 + 1:r0 + 9, 1:65])
                            nc.scalar.dma_start(
                                out=out_d[1][:, n * 512:(n + 1) * 512],
                                in_=V[0][0:64, n * 512:(n + 1) * 512])
                else:
                    D_next = E_next = G_next = None
                if p >= 2:
                    for n in range(NT):
                        emit_unit(1, p - 1, n)
                        if p == T + 1:
                            r0 = n * 8
                            nc.sync.dma_start(
                                out=o_h1[:, r0:r0 + 8, :],
                                in_=U1f[64:128, r0 + 1:r0 + 9, 1:65])
                            nc.scalar.dma_start(
                                out=out_d[3][:, n * 512:(n + 1) * 512],
                                in_=V[1][64:128, n * 512:(n + 1) * 512])
                D_cur, E_cur, G_cur = D_next, E_next, G_next

    nc.compile()
    _EMITTED["nc"] = nc
    return nc


def kernel(x, Wx0, Wh0, b0, Wx1, Wh1, b1):
    from concourse.bass_utils import run_bass_kernel_spmd

    nc = _emit()
    packed = pack_weights(np.asarray(Wx0), np.asarray(Wh0), np.asarray(b0),
                          np.asarray(Wx1), np.asarray(Wh1), np.asarray(b1))
    x = np.asarray(x)
    in_maps = []
    for b in range(B):
        m = dict(packed)
        m["x9"] = build_x9(x[b, :, :, :, 0])
        m["zeros"] = np.zeros((128, NPAD), np.float32)
        in_maps.append(m)
    res = run_bass_kernel_spmd(nc, in_maps, list(range(B)))

    out = np.zeros((2, 2, B, H, W, F), np.float32)
    for b in range(B):
        r = res.results[b]["out"]  # [4, 64, 4096]
        for (l, s), arr in (((0, 0), r[0]), ((0, 1), r[1]),
                            ((1, 0), r[2]), ((1, 1), r[3])):
            out[l, s, b] = arr.T.reshape(H, W, F)
    return out
